# revision 11
# baseline (speedup 1.0000x reference)
"""Trainium2 Bass kernel for BasisOrbitalBackflow.

Math: for each batch b,
    basis[i, k*NB+l] = (1/(N-1)) * (S[k] - chi[i,k]) * chi[i,l],  S = chi.sum(axis=0)
    out = basis @ W
(the mean over j != i of the pair outer product chi[j,k]*chi[i,l] collapses to
an outer product of the "leave-one-out" column sum with the row itself).

Sharding: data parallel, batch b -> core b (B == n_cores == 8).

Per-core dataflow (all shapes [partition, free]):
  chi [128i, 32k] --matmul(ones)--> S_row [1, 32] --matmul(ones_row)--> S_bcast [128i, 32k]
  u = S_bcast - chi                                      (DVE)
  urep chunks c=0..7: urep_c[p, i] = u[i, 4c + p>>5]     (PE: col-repeat lhsT x identity)
  vrep[p, i] = chi[i, p & 31]                            (PE: col-tile lhsT x identity)
  bT[p, c*128+i] = urep_c[p, i] * vrep[p, i]             (DVE, one [128,1024] mult)
  out[i, o] = sum_c sum_p bT_c[p, i] * w2[p, c*128+o]    (PE: 8 accumulating matmuls)
where w2[p, c*128+o] = W[c*128+p, o] / (N-1)  (host-side repack of backflow_coeff).
"""

import numpy as np

B, N, NB, NORB = 8, 128, 32, 128
NB2 = NB * NB  # 1024
NCHUNK = NB2 // 128  # 8

_cache = {}


def _build():
    import concourse.bacc as bacc
    import concourse.mybir as mybir
    from concourse.tile import TileContext

    f32 = mybir.dt.float32
    nc = bacc.Bacc(None, target_bir_lowering=False, debug=False, num_devices=8)
    chi = nc.declare_dram_parameter("chi", [N, NB], f32, isOutput=False)
    w2 = nc.declare_dram_parameter("w2", [128, NB2], f32, isOutput=False)
    y = nc.declare_dram_parameter("y", [N, NORB], f32, isOutput=True)

    with TileContext(nc) as tc:
        with (
            tc.tile_pool(name="sb", bufs=1) as pool,
            tc.tile_pool(name="ps", bufs=1, space="PSUM") as psum,
        ):
            # chi on the sync HWDGE ring (critical path), w2 on the scalar ring
            # so the big transfer does not queue ahead of the small one.
            t_chi = pool.tile([N, NB], f32)
            nc.sync.dma_start(out=t_chi[:], in_=chi[:])
            t_w2 = pool.tile([128, NB2], f32)
            nc.scalar.dma_start(out=t_w2[:], in_=w2[:])

            # PE warm-up: the HAM clock gate keeps the PE at 1.2 GHz until it has
            # been busy ~3.4us. The PE is otherwise idle while the chi DMA is in
            # flight, so burn that window on dummy matmuls to enter the real
            # work at 2.4 GHz.
            t_garb = pool.tile([128, 512], f32)
            nc.vector.memset(t_garb[:], 1.0)
            ps_warm = psum.tile([128, 512], f32)
            for _ in range(3):
                nc.tensor.matmul(
                    ps_warm[:], lhsT=t_garb[:, :128], rhs=t_garb[:], start=True, stop=True
                )

            # constants built on-device
            t_fill = pool.tile([128, 128], f32)
            nc.gpsimd.memset(t_fill[:], 1.0)
            t_ident = pool.tile([128, 128], f32)
            nc.gpsimd.affine_select(
                out=t_ident[:],
                in_=t_fill[:],
                pattern=[[1, 128]],
                compare_op=mybir.AluOpType.is_equal,
                fill=0.0,
                base=0,
                channel_multiplier=-1,
            )
            # ONES - I: contracting chi with this directly yields the
            # leave-one-out column sums u[i,k] = sum_{j != i} chi[j,k]
            t_offdiag = pool.tile([128, 128], f32)
            nc.gpsimd.affine_select(
                out=t_offdiag[:],
                in_=t_fill[:],
                pattern=[[1, 128]],
                compare_op=mybir.AluOpType.not_equal,
                fill=0.0,
                base=0,
                channel_multiplier=-1,
            )

            # t_vr[i', a*32 + l] = chi[i', l]  (chi tiled 4x along free)
            t_vr = pool.tile([N, 128], f32)
            nc.scalar.activation(
                t_vr[:, :].rearrange("p (a l) -> p a l", a=4),
                t_chi[:, :].rearrange("p (one l) -> p one l", one=1).broadcast_to([N, 4, NB]),
                mybir.ActivationFunctionType.Copy,
            )

            # u[i, k] = sum_{j != i} chi[j, k]  (one matmul; offdiag is symmetric)
            ps_u = psum.tile([N, NB], f32)
            nc.tensor.matmul(ps_u[:], lhsT=t_offdiag[:], rhs=t_chi[:], start=True, stop=True)
            t_u = pool.tile([N, NB], f32)
            nc.vector.tensor_copy(t_u[:], ps_u[:])

            # vrep[p, i] = chi[i, p & 31]  (PE transpose mode: fp32 single pass)
            ps_vrep = psum.tile([128, 128], f32)
            nc.tensor.transpose(ps_vrep[:], t_vr[:], t_ident[:])
            t_vrep = pool.tile([128, 128], f32)
            nc.scalar.activation(t_vrep[:], ps_vrep[:], mybir.ActivationFunctionType.Copy)

            # chunk-level pipeline: per chunk c
            #   t_ur_c[i', 32t + r] = u[i', 4c + t]      (DVE/ACT stride-0 copy)
            #   urep_c = t_ur_c.T                        (PE transpose, ping-pong PSUM)
            #   bT_c = urep_c * vrep                     (DVE)
            #   ps_out += bT_c.T @ w2_c                  (PE, accumulating)
            t_ur = pool.tile([N, NB2], f32)
            t_bT = pool.tile([128, NB2], f32)
            ps_out = psum.tile([N, NORB], f32)
            ps_pingA = psum.tile([128, 128], f32, name="ps_pingA")
            ps_pingB = psum.tile([128, 128], f32, name="ps_pingB")
            ps_ping = [ps_pingA, ps_pingB]
            for c in range(NCHUNK):
                ur_c = t_ur[:, c * 128 : (c + 1) * 128]
                src_c = (
                    t_u[:, 4 * c : 4 * c + 4]
                    .rearrange("p (j one) -> p j one", one=1)
                    .broadcast_to([N, 4, 32])
                )
                if c % 2 == 0:
                    nc.vector.tensor_copy(ur_c.rearrange("p (j r) -> p j r", j=4), src_c)
                else:
                    nc.scalar.activation(
                        ur_c.rearrange("p (j r) -> p j r", j=4),
                        src_c,
                        mybir.ActivationFunctionType.Copy,
                    )
                ps_c = ps_ping[c % 2]
                nc.tensor.transpose(ps_c[:], ur_c, t_ident[:])
                bT_c = t_bT[:, c * 128 : (c + 1) * 128]
                nc.vector.tensor_mul(bT_c, ps_c[:], t_vrep[:])
                nc.tensor.matmul(
                    ps_out[:],
                    lhsT=bT_c,
                    rhs=t_w2[:, c * NORB : (c + 1) * NORB],
                    start=(c == 0),
                    stop=(c == NCHUNK - 1),
                )
            t_out = pool.tile([N, NORB], f32)
            nc.vector.tensor_copy(t_out[:], ps_out[:])
            nc.sync.dma_start(out=y[:], in_=t_out[:])

    nc.compile()
    return nc


def get_nc():
    if "nc" not in _cache:
        _cache["nc"] = _build()
    return _cache["nc"]


def make_in_maps(chi, backflow_coeff):
    chi = np.ascontiguousarray(chi, dtype=np.float32)
    w = np.ascontiguousarray(backflow_coeff, dtype=np.float32)
    w2 = (w / np.float32(N - 1)).reshape(NCHUNK, 128, NORB).transpose(1, 0, 2)
    w2 = np.ascontiguousarray(w2.reshape(128, NCHUNK * NORB))
    return [{"chi": chi[b], "w2": w2} for b in range(B)]


def kernel(chi, backflow_coeff):
    from concourse.bass_utils import run_bass_kernel_spmd

    nc = get_nc()
    in_maps = make_in_maps(chi, backflow_coeff)
    res = run_bass_kernel_spmd(nc, in_maps, list(range(B)))
    return np.stack([res.results[b]["y"] for b in range(B)])


# revision 13
# speedup vs baseline: 1.0435x; 1.0435x over previous
"""Trainium2 Bass kernel for BasisOrbitalBackflow.

Math: for each batch b,
    basis[i, k*NB+l] = (1/(N-1)) * (S[k] - chi[i,k]) * chi[i,l],  S = chi.sum(axis=0)
    out = basis @ W
(the mean over j != i of the pair outer product chi[j,k]*chi[i,l] collapses to
an outer product of the "leave-one-out" column sum with the row itself).

Sharding: data parallel, batch b -> core b (B == n_cores == 8).

Per-core dataflow (all shapes [partition, free]):
  chi [128i, 32k] --matmul(ones)--> S_row [1, 32] --matmul(ones_row)--> S_bcast [128i, 32k]
  u = S_bcast - chi                                      (DVE)
  urep chunks c=0..7: urep_c[p, i] = u[i, 4c + p>>5]     (PE: col-repeat lhsT x identity)
  vrep[p, i] = chi[i, p & 31]                            (PE: col-tile lhsT x identity)
  bT[p, c*128+i] = urep_c[p, i] * vrep[p, i]             (DVE, one [128,1024] mult)
  out[i, o] = sum_c sum_p bT_c[p, i] * w2[p, c*128+o]    (PE: 8 accumulating matmuls)
where w2[p, c*128+o] = W[c*128+p, o] / (N-1)  (host-side repack of backflow_coeff).
"""

import numpy as np

B, N, NB, NORB = 8, 128, 32, 128
NWARM = 2
NB2 = NB * NB  # 1024
NCHUNK = NB2 // 128  # 8

_cache = {}


def _build():
    import concourse.bacc as bacc
    import concourse.mybir as mybir
    from concourse.tile import TileContext

    f32 = mybir.dt.float32
    nc = bacc.Bacc(None, target_bir_lowering=False, debug=False, num_devices=8)
    chi = nc.declare_dram_parameter("chi", [N, NB], f32, isOutput=False)
    w2 = nc.declare_dram_parameter("w2", [128, NB2], f32, isOutput=False)
    y = nc.declare_dram_parameter("y", [N, NORB], f32, isOutput=True)

    with TileContext(nc) as tc:
        with (
            tc.tile_pool(name="sb", bufs=1) as pool,
            tc.tile_pool(name="ps", bufs=1, space="PSUM") as psum,
        ):
            # chi on the sync HWDGE ring (critical path), w2 on the scalar ring
            # so the big transfer does not queue ahead of the small one.
            t_chi = pool.tile([N, NB], f32)
            nc.sync.dma_start(out=t_chi[:], in_=chi[:])
            t_w2 = pool.tile([128, NB2], f32)
            nc.scalar.dma_start(out=t_w2[:], in_=w2[:])

            # PE warm-up: the HAM clock gate keeps the PE at 1.2 GHz until it has
            # been busy ~3.4us. The PE is otherwise idle while the chi DMA is in
            # flight, so burn that window on dummy matmuls to enter the real
            # work at 2.4 GHz.
            t_garb = pool.tile([128, 128], f32)
            nc.vector.memset(t_garb[:], 1.0)
            ps_warm = psum.tile([128, 128], f32)
            for _ in range(NWARM):
                nc.tensor.matmul(
                    ps_warm[:], lhsT=t_garb[:], rhs=t_garb[:], start=True, stop=True
                )

            # constants built on-device
            t_fill = pool.tile([128, 128], f32)
            nc.gpsimd.memset(t_fill[:], 1.0)
            t_ident = pool.tile([128, 128], f32)
            nc.gpsimd.affine_select(
                out=t_ident[:],
                in_=t_fill[:],
                pattern=[[1, 128]],
                compare_op=mybir.AluOpType.is_equal,
                fill=0.0,
                base=0,
                channel_multiplier=-1,
            )
            # ONES - I: contracting chi with this directly yields the
            # leave-one-out column sums u[i,k] = sum_{j != i} chi[j,k]
            t_offdiag = pool.tile([128, 128], f32)
            nc.gpsimd.affine_select(
                out=t_offdiag[:],
                in_=t_fill[:],
                pattern=[[1, 128]],
                compare_op=mybir.AluOpType.not_equal,
                fill=0.0,
                base=0,
                channel_multiplier=-1,
            )

            # t_vr[i', a*32 + l] = chi[i', l]  (chi tiled 4x along free)
            t_vr = pool.tile([N, 128], f32)
            nc.scalar.activation(
                t_vr[:, :].rearrange("p (a l) -> p a l", a=4),
                t_chi[:, :].rearrange("p (one l) -> p one l", one=1).broadcast_to([N, 4, NB]),
                mybir.ActivationFunctionType.Copy,
            )

            # u[i, k] = sum_{j != i} chi[j, k]  (one matmul; offdiag is symmetric)
            ps_u = psum.tile([N, NB], f32)
            nc.tensor.matmul(ps_u[:], lhsT=t_offdiag[:], rhs=t_chi[:], start=True, stop=True)
            t_u = pool.tile([N, NB], f32)
            nc.vector.tensor_copy(t_u[:], ps_u[:])

            # vrep[p, i] = chi[i, p & 31]  (PE transpose mode: fp32 single pass)
            ps_vrep = psum.tile([128, 128], f32)
            nc.tensor.transpose(ps_vrep[:], t_vr[:], t_ident[:])
            t_vrep = pool.tile([128, 128], f32)
            nc.scalar.activation(t_vrep[:], ps_vrep[:], mybir.ActivationFunctionType.Copy)

            # chunk-level pipeline: per chunk c
            #   t_ur_c[i', 32t + r] = u[i', 4c + t]      (DVE/ACT stride-0 copy)
            #   urep_c = t_ur_c.T                        (PE transpose, ping-pong PSUM)
            #   bT_c = urep_c * vrep                     (DVE)
            #   ps_out += bT_c.T @ w2_c                  (PE, accumulating)
            t_ur = pool.tile([N, NB2], f32)
            t_bT = pool.tile([128, NB2], f32)
            ps_out = psum.tile([N, NORB], f32)
            ps_pingA = psum.tile([128, 128], f32, name="ps_pingA")
            ps_pingB = psum.tile([128, 128], f32, name="ps_pingB")
            ps_ping = [ps_pingA, ps_pingB]
            for c in range(NCHUNK):
                ur_c = t_ur[:, c * 128 : (c + 1) * 128]
                src_c = (
                    t_u[:, 4 * c : 4 * c + 4]
                    .rearrange("p (j one) -> p j one", one=1)
                    .broadcast_to([N, 4, 32])
                )
                if c % 2 == 0:
                    nc.vector.tensor_copy(ur_c.rearrange("p (j r) -> p j r", j=4), src_c)
                else:
                    nc.scalar.activation(
                        ur_c.rearrange("p (j r) -> p j r", j=4),
                        src_c,
                        mybir.ActivationFunctionType.Copy,
                    )
                ps_c = ps_ping[c % 2]
                nc.tensor.transpose(ps_c[:], ur_c, t_ident[:])
                bT_c = t_bT[:, c * 128 : (c + 1) * 128]
                nc.vector.tensor_mul(bT_c, ps_c[:], t_vrep[:])
                nc.tensor.matmul(
                    ps_out[:],
                    lhsT=bT_c,
                    rhs=t_w2[:, c * NORB : (c + 1) * NORB],
                    start=(c == 0),
                    stop=(c == NCHUNK - 1),
                )
            t_out = pool.tile([N, NORB], f32)
            nc.vector.tensor_copy(t_out[:], ps_out[:])
            nc.sync.dma_start(out=y[:], in_=t_out[:])

    nc.compile()
    return nc


def get_nc():
    if "nc" not in _cache:
        _cache["nc"] = _build()
    return _cache["nc"]


def make_in_maps(chi, backflow_coeff):
    chi = np.ascontiguousarray(chi, dtype=np.float32)
    w = np.ascontiguousarray(backflow_coeff, dtype=np.float32)
    w2 = (w / np.float32(N - 1)).reshape(NCHUNK, 128, NORB).transpose(1, 0, 2)
    w2 = np.ascontiguousarray(w2.reshape(128, NCHUNK * NORB))
    return [{"chi": chi[b], "w2": w2} for b in range(B)]


def kernel(chi, backflow_coeff):
    from concourse.bass_utils import run_bass_kernel_spmd

    nc = get_nc()
    in_maps = make_in_maps(chi, backflow_coeff)
    res = run_bass_kernel_spmd(nc, in_maps, list(range(B)))
    return np.stack([res.results[b]["y"] for b in range(B)])


# revision 15
# speedup vs baseline: 1.2021x; 1.1519x over previous
"""Trainium2 Bass kernel for BasisOrbitalBackflow.

Math: for each batch b,
    basis[i, k*NB+l] = (1/(N-1)) * (S[k] - chi[i,k]) * chi[i,l],  S = chi.sum(axis=0)
    out = basis @ W
(the mean over j != i of the pair outer product chi[j,k]*chi[i,l] collapses to
an outer product of the "leave-one-out" column sum with the row itself).

Sharding: data parallel, batch b -> core b (B == n_cores == 8).

Per-core dataflow (all shapes [partition, free]):
  chi [128i, 32k] --matmul(ones)--> S_row [1, 32] --matmul(ones_row)--> S_bcast [128i, 32k]
  u = S_bcast - chi                                      (DVE)
  urep chunks c=0..7: urep_c[p, i] = u[i, 4c + p>>5]     (PE: col-repeat lhsT x identity)
  vrep[p, i] = chi[i, p & 31]                            (PE: col-tile lhsT x identity)
  bT[p, c*128+i] = urep_c[p, i] * vrep[p, i]             (DVE, one [128,1024] mult)
  out[i, o] = sum_c sum_p bT_c[p, i] * w2[p, c*128+o]    (PE: 8 accumulating matmuls)
where w2[p, c*128+o] = W[c*128+p, o] / (N-1)  (host-side repack of backflow_coeff).
"""

import numpy as np

B, N, NB, NORB = 8, 128, 32, 128
NWARM = 0
# "fp32": full precision everywhere (rel err ~1e-6).
# "fp16": basis/W contraction in fp16 (single-pass PE, FWL) — rel err ~5e-4.
CONTRACT = "fp16"

NB2 = NB * NB  # 1024
NCHUNK = NB2 // 128  # 8

_cache = {}


def _build():
    import concourse.bacc as bacc
    import concourse.mybir as mybir
    from concourse.tile import TileContext

    f32 = mybir.dt.float32
    fc = mybir.dt.float16 if CONTRACT == "fp16" else f32
    nc = bacc.Bacc(None, target_bir_lowering=False, debug=False, num_devices=8)
    chi = nc.declare_dram_parameter("chi", [N, NB], f32, isOutput=False)
    w2 = nc.declare_dram_parameter("w2", [128, NB2], fc, isOutput=False)
    y = nc.declare_dram_parameter("y", [N, NORB], f32, isOutput=True)

    with TileContext(nc) as tc:
        with (
            tc.tile_pool(name="sb", bufs=1) as pool,
            tc.tile_pool(name="ps", bufs=1, space="PSUM") as psum,
        ):
            # chi on the sync HWDGE ring (critical path), w2 on the scalar ring
            # so the big transfer does not queue ahead of the small one.
            t_chi = pool.tile([N, NB], f32)
            nc.sync.dma_start(out=t_chi[:], in_=chi[:])
            t_w2 = pool.tile([128, NB2], fc)
            nc.scalar.dma_start(out=t_w2[:], in_=w2[:])

            # PE warm-up: the HAM clock gate keeps the PE at 1.2 GHz until it has
            # been busy ~3.4us. The PE is otherwise idle while the chi DMA is in
            # flight, so burn that window on dummy matmuls to enter the real
            # work at 2.4 GHz.
            t_garb = pool.tile([128, 128], f32)
            nc.vector.memset(t_garb[:], 1.0)
            ps_warm = psum.tile([128, 128], f32)
            for _ in range(NWARM):
                nc.tensor.matmul(
                    ps_warm[:], lhsT=t_garb[:], rhs=t_garb[:], start=True, stop=True
                )

            # constants built on-device
            t_fill = pool.tile([128, 128], f32)
            nc.gpsimd.memset(t_fill[:], 1.0)
            t_identc = pool.tile([128, 128], fc)
            t_fillc = pool.tile([128, 128], fc)
            nc.gpsimd.memset(t_fillc[:], 1.0)
            nc.gpsimd.affine_select(
                out=t_identc[:],
                in_=t_fillc[:],
                pattern=[[1, 128]],
                compare_op=mybir.AluOpType.is_equal,
                fill=0.0,
                base=0,
                channel_multiplier=-1,
            )
            # ONES - I: contracting chi with this directly yields the
            # leave-one-out column sums u[i,k] = sum_{j != i} chi[j,k]
            t_offdiag = pool.tile([128, 128], f32)
            nc.gpsimd.affine_select(
                out=t_offdiag[:],
                in_=t_fill[:],
                pattern=[[1, 128]],
                compare_op=mybir.AluOpType.not_equal,
                fill=0.0,
                base=0,
                channel_multiplier=-1,
            )

            # t_vr[i', a*32 + l] = chi[i', l]  (chi tiled 4x along free)
            t_vr = pool.tile([N, 128], fc)
            nc.scalar.activation(
                t_vr[:, :].rearrange("p (a l) -> p a l", a=4),
                t_chi[:, :].rearrange("p (one l) -> p one l", one=1).broadcast_to([N, 4, NB]),
                mybir.ActivationFunctionType.Copy,
            )

            # u[i, k] = sum_{j != i} chi[j, k]  (one matmul; offdiag is symmetric)
            ps_u = psum.tile([N, NB], f32)
            nc.tensor.matmul(ps_u[:], lhsT=t_offdiag[:], rhs=t_chi[:], start=True, stop=True)
            t_u = pool.tile([N, NB], f32)
            nc.vector.tensor_copy(t_u[:], ps_u[:])

            # vrep[p, i] = chi[i, p & 31]  (PE transpose mode: fp32 single pass)
            ps_vrep = psum.tile([128, 128], fc)
            nc.tensor.transpose(ps_vrep[:], t_vr[:], t_identc[:])
            t_vrep = pool.tile([128, 128], fc)
            nc.scalar.activation(t_vrep[:], ps_vrep[:], mybir.ActivationFunctionType.Copy)

            # chunk-level pipeline: per chunk c
            #   t_ur_c[i', 32t + r] = u[i', 4c + t]      (DVE/ACT stride-0 copy)
            #   urep_c = t_ur_c.T                        (PE transpose, ping-pong PSUM)
            #   bT_c = urep_c * vrep                     (DVE)
            #   ps_out += bT_c.T @ w2_c                  (PE, accumulating)
            t_ur = pool.tile([N, NB2], fc)
            t_bT = pool.tile([128, NB2], fc)
            ps_out = psum.tile([N, NORB], f32)
            ps_pingA = psum.tile([128, 128], fc, name="ps_pingA")
            ps_pingB = psum.tile([128, 128], fc, name="ps_pingB")
            ps_ping = [ps_pingA, ps_pingB]
            for c in range(NCHUNK):
                ur_c = t_ur[:, c * 128 : (c + 1) * 128]
                src_c = (
                    t_u[:, 4 * c : 4 * c + 4]
                    .rearrange("p (j one) -> p j one", one=1)
                    .broadcast_to([N, 4, 32])
                )
                if c % 2 == 0:
                    nc.vector.tensor_copy(ur_c.rearrange("p (j r) -> p j r", j=4), src_c)
                else:
                    nc.scalar.activation(
                        ur_c.rearrange("p (j r) -> p j r", j=4),
                        src_c,
                        mybir.ActivationFunctionType.Copy,
                    )
                ps_c = ps_ping[c % 2]
                nc.tensor.transpose(ps_c[:], ur_c, t_identc[:])
                bT_c = t_bT[:, c * 128 : (c + 1) * 128]
                nc.vector.tensor_mul(bT_c, ps_c[:], t_vrep[:])
                nc.tensor.matmul(
                    ps_out[:],
                    lhsT=bT_c,
                    rhs=t_w2[:, c * NORB : (c + 1) * NORB],
                    start=(c == 0),
                    stop=(c == NCHUNK - 1),
                )
            t_out = pool.tile([N, NORB], f32)
            nc.vector.tensor_copy(t_out[:], ps_out[:])
            nc.sync.dma_start(out=y[:], in_=t_out[:])

    nc.compile()
    return nc


def get_nc():
    if "nc" not in _cache:
        _cache["nc"] = _build()
    return _cache["nc"]


def make_in_maps(chi, backflow_coeff):
    chi = np.ascontiguousarray(chi, dtype=np.float32)
    w = np.ascontiguousarray(backflow_coeff, dtype=np.float32)
    w2 = (w / np.float32(N - 1)).reshape(NCHUNK, 128, NORB).transpose(1, 0, 2)
    w2 = w2.reshape(128, NCHUNK * NORB)
    if CONTRACT == "fp16":
        w2 = w2.astype(np.float16)
    w2 = np.ascontiguousarray(w2)
    return [{"chi": chi[b], "w2": w2} for b in range(B)]


def kernel(chi, backflow_coeff):
    from concourse.bass_utils import run_bass_kernel_spmd

    nc = get_nc()
    in_maps = make_in_maps(chi, backflow_coeff)
    res = run_bass_kernel_spmd(nc, in_maps, list(range(B)))
    return np.stack([res.results[b]["y"] for b in range(B)])
